# revision 1
# baseline (speedup 1.0000x reference)
"""Trainium2 Bass kernel: parity-polynomial segment_reduce.

Reference math:
    spins = 1 - 2*bits                                   # {-1,+1}
    parities[b,t] = prod_o spins_pad[b, idx_pad[t,o]]    # [B, T]
    out[b] = parities[b] @ theta

Every parity factor is (-1)^{bit}, so
    out[b] = sum_t theta[t] * (-1)^{popcount(key[b] & mask[t])}
with key[b] = sum_i bits[b,i]<<i and mask[t] = XOR-fold of (1<<idx_pad[t,o])
(the pad index NUM_BITS maps to a constant-one column, i.e. contributes no bit;
a repeated index squares to +1, which XOR-folding reproduces).

For this problem idx_pad only references bits 0..11, so every mask < 4096 and
out[b] = f(key12[b]) where f = WHT_4096(theta_spread) — a 4096-point
Walsh-Hadamard transform of theta scattered by mask.  On device (per core,
batch-sharded 512 rows):

  1. WHT via the Kronecker split H_4096 = H_128 (x) H_32 (fp32, tiny):
         F[p,c] = (H128 @ Theta @ H32)[p,c],  Theta[q,d] = theta_spread[q*32+d]
  2. Per-row keys minus partition index in one bf16 matmul: bitsT carries a
     constant-ones row whose stationary weight is -j, so PSUM gets
     key(b) - j exactly (all operands are bf16-exact small integers; PSUM
     accumulates fp32).  bf16 runs the PE at full rate (4x fp32).
  3. One-hots via is_equal against immediate 0.0 (the pointer-scalar
     tensor_scalar form has no sync-wait slot in the ISA).
  4. Gather F rows with a one-hot matmul, mask columns with the c one-hot,
     column-reduce with a ones-vector matmul:
         out[b] = sum_c (F^T @ onehot_p)[c,b] * onehot_c[c,b] = F[p_b, c_b].

Sync-slot discipline (walrus "Too many sync wait commands"): inputs are packed
into 3 DMAs, every PSUM->SBUF staging copy runs on DVE only, and a warm-up
matmul lets PE observe each DMA semaphore before the consuming matmul, so no
instruction ever needs more than one new cross-engine wait.

Host does only sharding, dtype/layout staging, and the index bookkeeping
(mask XOR-fold + theta scatter).  All theta- and bit-dependent arithmetic
runs on device.
"""

import numpy as np

B, NUM_BITS, ORDER = 4096, 32, 12
N_CORES = 8
B_LOCAL = B // N_CORES          # 512
KEYS = 1 << ORDER               # 4096
P_DIM, C_DIM = 128, 32          # KEYS = P_DIM * C_DIM ; p = key>>5, c = key&31
P_BITS, C_BITS = 7, 5
ROWS = NUM_BITS + 1             # bits rows + constant-ones row
BB_COLS = B_LOCAL + P_DIM + C_DIM   # bitsT | wp_aug | wc_aug   (bf16 pack)
PKF_COLS = P_DIM + C_DIM            # thetaT | h32               (fp32 pack)

_STATE = {}


def _sylvester(n):
    """H[i,j] = (-1)^popcount(i&j), Sylvester ordering."""
    h = np.array([[1.0]], dtype=np.float32)
    while h.shape[0] < n:
        h = np.block([[h, h], [h, -h]])
    return np.ascontiguousarray(h, dtype=np.float32)


def _build_module():
    import concourse.mybir as mybir
    import concourse.tile as tile
    from concourse import bacc

    f32 = mybir.dt.float32
    bf16 = mybir.dt.bfloat16
    nc = bacc.Bacc(
        "TRN2",
        target_bir_lowering=False,
        debug=False,
        enable_asserts=True,
        num_devices=N_CORES,
    )

    bb = nc.dram_tensor("bb", [ROWS, BB_COLS], bf16, kind="ExternalInput").ap()
    pkf = nc.dram_tensor("pkf", [C_DIM, PKF_COLS], f32, kind="ExternalInput").ap()
    pk128 = nc.dram_tensor("pk128", [P_DIM, P_DIM], f32, kind="ExternalInput").ap()
    out = nc.dram_tensor("out", [1, B_LOCAL], f32, kind="ExternalOutput").ap()

    with tile.TileContext(nc) as tc:
        with (
            tc.tile_pool(name="sb", bufs=1) as sb,
            tc.tile_pool(name="ps", bufs=1, space="PSUM") as ps,
        ):
            t_bb = sb.tile([ROWS, BB_COLS], bf16)
            nc.sync.dma_start(out=t_bb, in_=bb)
            t_pkf = sb.tile([C_DIM, PKF_COLS], f32)
            nc.sync.dma_start(out=t_pkf, in_=pkf)
            t_pk128 = sb.tile([P_DIM, P_DIM], f32)
            nc.sync.dma_start(out=t_pk128, in_=pk128)

            t_bitsT = t_bb[:, 0:B_LOCAL]
            t_wp = t_bb[:, B_LOCAL : B_LOCAL + P_DIM]
            t_wc = t_bb[:, B_LOCAL + P_DIM : B_LOCAL + P_DIM + C_DIM]
            t_thetaT = t_pkf[:, 0:P_DIM]
            t_h32 = t_pkf[:, P_DIM : P_DIM + C_DIM]
            t_h128 = t_pk128

            t_ones = sb.tile([C_DIM, 1], bf16)
            nc.vector.memset(t_ones, 1.0)

            # --- WHT of theta_spread: F = H128 @ Theta @ H32 (fp32) ---
            p_G = ps.tile([P_DIM, C_DIM], f32)
            nc.tensor.matmul(p_G, t_thetaT, t_h32)          # waits: pkf DMA
            p_warm = ps.tile([1, 1], f32)
            nc.tensor.matmul(p_warm, t_h128[:, 0:1], t_h128[:, 0:1])  # waits: pk128
            t_G = sb.tile([P_DIM, C_DIM], f32)
            nc.vector.tensor_copy(t_G, p_G)
            p_F = ps.tile([P_DIM, C_DIM], f32)
            nc.tensor.matmul(p_F, t_h128, t_G)              # waits: DVE only
            t_F = sb.tile([P_DIM, C_DIM], bf16)
            nc.vector.tensor_copy(t_F, p_F)                 # fp32 -> bf16

            # --- keys minus partition index (bf16 in, exact fp32 accum) ---
            p_bp = ps.tile([P_DIM, B_LOCAL], f32)
            nc.tensor.matmul(p_bp, t_wp, t_bitsT)  # [j, b] = p_key(b) - j
            t_ohp = sb.tile([P_DIM, B_LOCAL], bf16)
            nc.vector.tensor_scalar(
                out=t_ohp,
                in0=p_bp,
                scalar1=0.0,
                scalar2=None,
                op0=mybir.AluOpType.is_equal,
            )
            p_bc = ps.tile([C_DIM, B_LOCAL], f32)
            nc.tensor.matmul(p_bc, t_wc, t_bitsT)  # [j, b] = c_key(b) - j
            t_ohc = sb.tile([C_DIM, B_LOCAL], bf16)
            nc.vector.tensor_scalar(
                out=t_ohc,
                in0=p_bc,
                scalar1=0.0,
                scalar2=None,
                op0=mybir.AluOpType.is_equal,
            )

            # --- gather + reduce:  out[b] = F[p_b, c_b] ---
            p_o1 = ps.tile([C_DIM, B_LOCAL], f32)
            nc.tensor.matmul(p_o1, t_F, t_ohp)      # o1[c,b] = F[p_b, c]
            t_prod = sb.tile([C_DIM, B_LOCAL], bf16)
            nc.vector.tensor_mul(t_prod, p_o1, t_ohc)
            p_out = ps.tile([1, B_LOCAL], f32)
            nc.tensor.matmul(p_out, t_ones, t_prod)  # column sums
            t_out = sb.tile([1, B_LOCAL], f32)
            nc.vector.tensor_copy(t_out, p_out)
            nc.sync.dma_start(out=out, in_=t_out)

    nc.compile()
    return nc


def _get_module():
    nc = _STATE.get("nc")
    if nc is None:
        nc = _build_module()
        _STATE["nc"] = nc
    return nc


def _host_prep(bitstrings, theta, idx_pad):
    """Index bookkeeping + input staging. Returns per-core input maps."""
    import ml_dtypes

    bitstrings = np.asarray(bitstrings)
    theta = np.asarray(theta, dtype=np.float32)
    idx_pad = np.asarray(idx_pad).astype(np.int64)

    # mask[t] = XOR-fold of one-hot bit positions (pad index >= NUM_BITS -> no bit)
    onehots = np.where(idx_pad >= NUM_BITS, 0, np.int64(1) << np.clip(idx_pad, 0, 62))
    masks = np.bitwise_xor.reduce(onehots, axis=1)
    if masks.size and int(masks.max()) >= KEYS:
        raise NotImplementedError(
            "kernel specialized for masks spanning bits 0..11 "
            f"(max mask {int(masks.max())})"
        )
    theta_spread = np.zeros(KEYS, np.float32)
    np.add.at(theta_spread, masks, theta)

    # Stationary key weights; row 32 multiplies the constant-ones bit row,
    # its weight -j turns the matmul output into key(b) - j.
    wp = np.zeros((ROWS, P_DIM), np.float32)
    for k in range(C_BITS, ORDER):
        wp[k, :] = float(1 << (k - C_BITS))
    wp[NUM_BITS, :] = -np.arange(P_DIM, dtype=np.float32)
    wc = np.zeros((ROWS, C_DIM), np.float32)
    for k in range(C_BITS):
        wc[k, :] = float(1 << k)
    wc[NUM_BITS, :] = -np.arange(C_DIM, dtype=np.float32)

    pkf = np.zeros((C_DIM, PKF_COLS), np.float32)
    pkf[:, 0:P_DIM] = theta_spread.reshape(P_DIM, C_DIM).T
    pkf[:, P_DIM : P_DIM + C_DIM] = _sylvester(C_DIM)

    base = {"pkf": pkf, "pk128": _sylvester(P_DIM)}

    bits_f = bitstrings.astype(np.float32)
    in_maps = []
    for c in range(N_CORES):
        m = dict(base)
        bbuf = np.ones((ROWS, BB_COLS), np.float32)
        bbuf[:NUM_BITS, 0:B_LOCAL] = bits_f[c * B_LOCAL : (c + 1) * B_LOCAL, :].T
        bbuf[:, B_LOCAL : B_LOCAL + P_DIM] = wp
        bbuf[:, B_LOCAL + P_DIM :] = wc
        m["bb"] = bbuf.astype(ml_dtypes.bfloat16)
        in_maps.append(m)
    return in_maps


def kernel(bitstrings, theta, idx_pad):
    from concourse.bass_utils import run_bass_kernel_spmd

    in_maps = _host_prep(bitstrings, theta, idx_pad)
    nc = _get_module()
    res = run_bass_kernel_spmd(nc, in_maps, core_ids=list(range(N_CORES)))
    out = np.concatenate([np.asarray(r["out"][0]) for r in res.results])
    return out.astype(np.float32)



# revision 83
# speedup vs baseline: 1.0509x; 1.0509x over previous
"""Trainium2 Bass kernel: parity-polynomial segment_reduce.

Reference math:
    spins = 1 - 2*bits                                   # {-1,+1}
    parities[b,t] = prod_o spins_pad[b, idx_pad[t,o]]    # [B, T]
    out[b] = parities[b] @ theta

Every parity factor is (-1)^{bit}, so
    out[b] = sum_t theta[t] * (-1)^{popcount(key[b] & mask[t])}
with key[b] = sum_i bits[b,i]<<i and mask[t] = XOR-fold of (1<<idx_pad[t,o])
(the pad index NUM_BITS maps to a constant-one column, i.e. contributes no bit;
a repeated index squares to +1, which XOR-folding reproduces).

For this problem idx_pad only references bits 0..11, so every mask < 4096 and
out[b] = f(key12[b]) where f = WHT_4096(theta_spread) — a 4096-point
Walsh-Hadamard transform of theta scattered by mask.  On device (per core,
batch-sharded 512 rows):

  1. WHT via the Kronecker split H_4096 = H_128 (x) H_32 (fp32, tiny):
         F[p,c] = (H128 @ Theta @ H32)[p,c],  Theta[q,d] = theta_spread[q*32+d]
  2. Per-row keys minus partition index in one bf16 matmul: bitsT carries a
     constant-ones row whose stationary weight is -j, so PSUM gets
     key(b) - j exactly (all operands are bf16-exact small integers; PSUM
     accumulates fp32).  bf16 runs the PE at full rate (4x fp32).
  3. One-hots via is_equal against immediate 0.0 (the pointer-scalar
     tensor_scalar form has no sync-wait slot in the ISA).
  4. Gather F rows with a one-hot matmul, mask columns with the c one-hot,
     then column-reduce with FOUR 1-column ones-matmuls whose stationaries
     are 128-column slices of the product -- the batch lands on the output
     partitions, so the final PSUM->SBUF staging copy is [128, 4] instead
     of [1, 512] (~520ns less DVE time on the critical tail):
         out[i,g] = sum_c prod[c, g*128+i],  prod = (F^T@onehot_p)*onehot_c.

Sync-slot discipline (walrus "Too many sync wait commands"): inputs are packed
into 2 DMAs (each extra DMA also costs a serialized ~625ns HWDGE
descriptor-generation slot), every PSUM->SBUF staging copy runs on DVE only,
and a warm-up matmul lets PE observe each DMA semaphore before the consuming
matmul, so no instruction ever needs more than one new cross-engine wait.

Host does only sharding, dtype/layout staging, and the index bookkeeping
(mask XOR-fold + theta scatter).  All theta- and bit-dependent arithmetic
runs on device.
"""

import numpy as np

B, NUM_BITS, ORDER = 4096, 32, 12
N_CORES = 8
B_LOCAL = B // N_CORES          # 512
KEYS = 1 << ORDER               # 4096
P_DIM, C_DIM = 128, 32          # KEYS = P_DIM * C_DIM ; p = key>>5, c = key&31
P_BITS, C_BITS = 7, 5
ROWS = NUM_BITS + 1             # bits rows + constant-ones row
BB_COLS = B_LOCAL + P_DIM + C_DIM   # bitsT | wp_aug | wc_aug   (bf16 pack)
PKF_COLS = P_DIM + C_DIM            # thetaT | h32               (fp32 pack)

_STATE = {}


def _sylvester(n):
    """H[i,j] = (-1)^popcount(i&j), Sylvester ordering."""
    h = np.array([[1.0]], dtype=np.float32)
    while h.shape[0] < n:
        h = np.block([[h, h], [h, -h]])
    return np.ascontiguousarray(h, dtype=np.float32)


def _build_module():
    import concourse.mybir as mybir
    import concourse.tile as tile
    from concourse import bacc

    f32 = mybir.dt.float32
    bf16 = mybir.dt.bfloat16
    nc = bacc.Bacc(
        "TRN2",
        target_bir_lowering=False,
        debug=False,
        enable_asserts=True,
        num_devices=N_CORES,
    )

    bb = nc.dram_tensor("bb", [ROWS, BB_COLS], bf16, kind="ExternalInput").ap()
    # H128 | thetaT | h32 merged into ONE DMA: each extra DMA costs a full
    # serialized HWDGE descriptor-generation slot (~625ns)
    pkk = nc.dram_tensor(
        "pkk", [P_DIM, P_DIM + PKF_COLS], f32, kind="ExternalInput").ap()
    out = nc.dram_tensor("out", [128, 4], f32, kind="ExternalOutput").ap()

    with tile.TileContext(nc) as tc:
        with (
            tc.tile_pool(name="sb", bufs=1) as sb,
            tc.tile_pool(name="ps", bufs=1, space="PSUM") as ps,
        ):
            t_bb = sb.tile([ROWS, BB_COLS], bf16)
            nc.sync.dma_start(out=t_bb, in_=bb)
            t_pkk = sb.tile([P_DIM, P_DIM + PKF_COLS], f32)
            nc.sync.dma_start(out=t_pkk, in_=pkk)

            t_bitsT = t_bb[:, 0:B_LOCAL]
            t_wp = t_bb[:, B_LOCAL : B_LOCAL + P_DIM]
            t_wc = t_bb[:, B_LOCAL + P_DIM : B_LOCAL + P_DIM + C_DIM]
            t_h128 = t_pkk[:, 0:P_DIM]
            t_thetaT = t_pkk[0:C_DIM, P_DIM : P_DIM + P_DIM]
            t_h32 = t_pkk[0:C_DIM, P_DIM + P_DIM : P_DIM + P_DIM + C_DIM]

            t_ones = sb.tile([C_DIM, 1], bf16)
            nc.vector.memset(t_ones, 1.0)

            # --- WHT of theta_spread: F = H128 @ Theta @ H32 (fp32) ---
            p_G = ps.tile([P_DIM, C_DIM], f32)
            nc.tensor.matmul(p_G, t_thetaT, t_h32)          # waits: pkk DMA
            p_warm = ps.tile([1, 1], f32)
            nc.tensor.matmul(p_warm, t_h128[:, 0:1], t_h128[:, 0:1])
            t_G = sb.tile([P_DIM, C_DIM], f32)
            nc.vector.tensor_copy(t_G, p_G)
            p_F = ps.tile([P_DIM, C_DIM], f32)
            nc.tensor.matmul(p_F, t_h128, t_G)              # waits: DVE only
            t_F = sb.tile([P_DIM, C_DIM], bf16)
            nc.vector.tensor_copy(t_F, p_F)                 # fp32 -> bf16

            # --- keys minus partition index (bf16 in, exact fp32 accum) ---
            p_bp = ps.tile([P_DIM, B_LOCAL], f32)
            nc.tensor.matmul(p_bp, t_wp, t_bitsT)  # [j, b] = p_key(b) - j
            t_ohp = sb.tile([P_DIM, B_LOCAL], bf16)
            nc.vector.tensor_scalar(
                out=t_ohp,
                in0=p_bp,
                scalar1=0.0,
                scalar2=None,
                op0=mybir.AluOpType.is_equal,
            )
            p_bc = ps.tile([C_DIM, B_LOCAL], f32)
            nc.tensor.matmul(p_bc, t_wc, t_bitsT)  # [j, b] = c_key(b) - j
            t_ohc = sb.tile([C_DIM, B_LOCAL], bf16)
            nc.vector.tensor_scalar(
                out=t_ohc,
                in0=p_bc,
                scalar1=0.0,
                scalar2=None,
                op0=mybir.AluOpType.is_equal,
            )

            # --- gather + reduce:  out[b] = F[p_b, c_b] ---
            p_o1 = ps.tile([C_DIM, B_LOCAL], f32)
            nc.tensor.matmul(p_o1, t_F, t_ohp)      # o1[c,b] = F[p_b, c]
            t_prod = sb.tile([C_DIM, B_LOCAL], bf16)
            nc.vector.tensor_mul(t_prod, p_o1, t_ohc)
            # column sums, batch transposed onto output partitions: four
            # 1-column matmuls (stationary = a 128-column slice of prod)
            # leave only a [128, 4] PSUM->SBUF copy instead of [1, 512]
            p_out = ps.tile([128, 4], f32)
            for g in range(4):
                nc.tensor.matmul(
                    p_out[:, g : g + 1],
                    t_prod[:, g * 128 : (g + 1) * 128], t_ones)
            t_out = sb.tile([128, 4], f32)
            nc.vector.tensor_copy(t_out, p_out)
            nc.sync.dma_start(out=out, in_=t_out)

    nc.compile()
    return nc


def _get_module():
    nc = _STATE.get("nc")
    if nc is None:
        nc = _build_module()
        _STATE["nc"] = nc
    return nc


def _host_prep(bitstrings, theta, idx_pad):
    """Index bookkeeping + input staging. Returns per-core input maps."""
    import ml_dtypes

    bitstrings = np.asarray(bitstrings)
    theta = np.asarray(theta, dtype=np.float32)
    idx_pad = np.asarray(idx_pad).astype(np.int64)

    # mask[t] = XOR-fold of one-hot bit positions (pad index >= NUM_BITS -> no bit)
    onehots = np.where(idx_pad >= NUM_BITS, 0, np.int64(1) << np.clip(idx_pad, 0, 62))
    masks = np.bitwise_xor.reduce(onehots, axis=1)
    if masks.size and int(masks.max()) >= KEYS:
        raise NotImplementedError(
            "kernel specialized for masks spanning bits 0..11 "
            f"(max mask {int(masks.max())})"
        )
    theta_spread = np.zeros(KEYS, np.float32)
    np.add.at(theta_spread, masks, theta)

    # Stationary key weights; row 32 multiplies the constant-ones bit row,
    # its weight -j turns the matmul output into key(b) - j.
    wp = np.zeros((ROWS, P_DIM), np.float32)
    for k in range(C_BITS, ORDER):
        wp[k, :] = float(1 << (k - C_BITS))
    wp[NUM_BITS, :] = -np.arange(P_DIM, dtype=np.float32)
    wc = np.zeros((ROWS, C_DIM), np.float32)
    for k in range(C_BITS):
        wc[k, :] = float(1 << k)
    wc[NUM_BITS, :] = -np.arange(C_DIM, dtype=np.float32)

    pkk = np.zeros((P_DIM, P_DIM + PKF_COLS), np.float32)
    pkk[:, 0:P_DIM] = _sylvester(P_DIM)
    pkk[0:C_DIM, P_DIM : P_DIM + P_DIM] = theta_spread.reshape(P_DIM, C_DIM).T
    pkk[0:C_DIM, P_DIM + P_DIM : P_DIM + P_DIM + C_DIM] = _sylvester(C_DIM)

    base = {"pkk": pkk}

    bits_f = bitstrings.astype(np.float32)
    in_maps = []
    for c in range(N_CORES):
        m = dict(base)
        bbuf = np.ones((ROWS, BB_COLS), np.float32)
        bbuf[:NUM_BITS, 0:B_LOCAL] = bits_f[c * B_LOCAL : (c + 1) * B_LOCAL, :].T
        bbuf[:, B_LOCAL : B_LOCAL + P_DIM] = wp
        bbuf[:, B_LOCAL + P_DIM :] = wc
        m["bb"] = bbuf.astype(ml_dtypes.bfloat16)
        in_maps.append(m)
    return in_maps


def kernel(bitstrings, theta, idx_pad):
    from concourse.bass_utils import run_bass_kernel_spmd

    in_maps = _host_prep(bitstrings, theta, idx_pad)
    nc = _get_module()
    res = run_bass_kernel_spmd(nc, in_maps, core_ids=list(range(N_CORES)))
    # out[i, g] holds sample b_local = g*128 + i
    out = np.concatenate([np.asarray(r["out"]).T.ravel() for r in res.results])
    return out.astype(np.float32)



# revision 90
# speedup vs baseline: 1.0717x; 1.0198x over previous
"""Trainium2 Bass kernel: parity-polynomial segment_reduce.

Reference math:
    spins = 1 - 2*bits                                   # {-1,+1}
    parities[b,t] = prod_o spins_pad[b, idx_pad[t,o]]    # [B, T]
    out[b] = parities[b] @ theta

Every parity factor is (-1)^{bit}, so
    out[b] = sum_t theta[t] * (-1)^{popcount(key[b] & mask[t])}
with key[b] = sum_i bits[b,i]<<i and mask[t] = XOR-fold of (1<<idx_pad[t,o])
(the pad index NUM_BITS maps to a constant-one column, i.e. contributes no bit;
a repeated index squares to +1, which XOR-folding reproduces).

For this problem idx_pad only references bits 0..11, so every mask < 4096 and
out[b] = f(key12[b]) where f = WHT_4096(theta_spread) — a 4096-point
Walsh-Hadamard transform of theta scattered by mask.  On device (per core,
batch-sharded 512 rows):

  1. WHT via the 6/6 Kronecker split (p = key>>6, c = key&63):
         F64[p,c] = (H64 @ Ts @ H64)[p,c],  Ts[q,md] = theta_spread[q*64+md].
     The second stage uses an augmented stationary (zeros | H64) so F64
     lands on PSUM partitions 64:128, matching the p-one-hot rows.
  2. BOTH sub-key grids from ONE bf16 matmul stacked on 128 partitions
     (rows j<64: c(b)-j, rows 64+i: p(b)-i; the constant-ones bits row
     carries the -j offsets; all operands bf16-exact small integers, PSUM
     accumulates fp32), and ONE is_equal turns the stack into both one-hots
     -- this halves the key-matmul and one-hot cost of the 7/5 version.
  3. One-hot via is_equal against immediate 0.0 (the pointer-scalar
     tensor_scalar form has no sync-wait slot in the ISA).
  4. Gather F rows with a one-hot matmul, mask columns with the c one-hot,
     then column-reduce with FOUR 1-column ones-matmuls whose stationaries
     are 128-column slices of the product -- the batch lands on the output
     partitions, so the final PSUM->SBUF staging copy is [128, 4] instead
     of [1, 512] (~520ns less DVE time on the critical tail):
         out[i,g] = sum_c prod[c, g*128+i],  prod = (F64^T@onehot_p)*onehot_c.

Sync-slot discipline (walrus "Too many sync wait commands"): inputs are packed
into 2 DMAs (each extra DMA also costs a serialized ~625ns HWDGE
descriptor-generation slot), every PSUM->SBUF staging copy runs on DVE only,
and a warm-up matmul lets PE observe each DMA semaphore before the consuming
matmul, so no instruction ever needs more than one new cross-engine wait.

Host does only sharding, dtype/layout staging, and the index bookkeeping
(mask XOR-fold + theta scatter).  All theta- and bit-dependent arithmetic
runs on device.
"""

import numpy as np

B, NUM_BITS, ORDER = 4096, 32, 12
N_CORES = 8
B_LOCAL = B // N_CORES          # 512
KEYS = 1 << ORDER               # 4096
PC = 64                         # 6/6 split: p = key>>6, c = key&63
PC_BITS = 6
ROWS = NUM_BITS + 1             # bits rows + constant-ones row
BB_COLS = B_LOCAL + 2 * PC      # bitsT | W (c-grid 64 | p-grid 64)
PKK_COLS = 4 * PC               # thetaT64 | H64 | h64aug(zeros|H64)

_STATE = {}


def _sylvester(n):
    """H[i,j] = (-1)^popcount(i&j), Sylvester ordering."""
    h = np.array([[1.0]], dtype=np.float32)
    while h.shape[0] < n:
        h = np.block([[h, h], [h, -h]])
    return np.ascontiguousarray(h, dtype=np.float32)


def _build_module():
    import concourse.mybir as mybir
    import concourse.tile as tile
    from concourse import bacc

    f32 = mybir.dt.float32
    bf16 = mybir.dt.bfloat16
    nc = bacc.Bacc(
        "TRN2",
        target_bir_lowering=False,
        debug=False,
        enable_asserts=True,
        num_devices=N_CORES,
    )

    bb = nc.dram_tensor("bb", [ROWS, BB_COLS], bf16, kind="ExternalInput").ap()
    # thetaT64 | H64 | h64aug merged into ONE DMA: each extra DMA costs a
    # full serialized HWDGE descriptor-generation slot (~625ns)
    pkk = nc.dram_tensor(
        "pkk", [PC, PKK_COLS], f32, kind="ExternalInput").ap()
    out = nc.dram_tensor("out", [128, 4], f32, kind="ExternalOutput").ap()

    with tile.TileContext(nc) as tc:
        with (
            tc.tile_pool(name="sb", bufs=1) as sb,
            tc.tile_pool(name="ps", bufs=1, space="PSUM") as ps,
        ):
            t_bb = sb.tile([ROWS, BB_COLS], bf16)
            nc.sync.dma_start(out=t_bb, in_=bb)
            t_pkk = sb.tile([PC, PKK_COLS], f32)
            nc.sync.dma_start(out=t_pkk, in_=pkk)

            t_bitsT = t_bb[:, 0:B_LOCAL]
            t_W = t_bb[:, B_LOCAL : B_LOCAL + 2 * PC]
            t_thetaT = t_pkk[:, 0:PC]
            t_h64 = t_pkk[:, PC : 2 * PC]
            t_h64aug = t_pkk[:, 2 * PC : 4 * PC]   # [64, 128]: zeros | H64

            t_ones = sb.tile([PC, 1], bf16)
            nc.vector.memset(t_ones, 1.0)

            # --- WHT of theta_spread: F64 = H64 @ Ts @ H64 (fp32).  The
            # second stage uses an augmented stationary (zeros | H64) so F64
            # lands on PSUM partitions 64:128, matching the p-one-hot rows of
            # the stacked grid (matmul operands must share base partition).
            p_G = ps.tile([PC, PC], f32)
            nc.tensor.matmul(p_G, t_thetaT, t_h64)          # waits: pkk DMA
            p_warm = ps.tile([1, 1], f32)
            nc.tensor.matmul(p_warm, t_pkk[:, PC : PC + 1], t_pkk[:, PC : PC + 1])
            t_G = sb.tile([PC, PC], f32)
            nc.vector.tensor_copy(t_G, p_G)
            p_F = ps.tile([2 * PC, PC], f32)
            nc.tensor.matmul(p_F, t_h64aug, t_G)            # waits: DVE only
            t_F = sb.tile([2 * PC, PC], bf16)
            nc.vector.tensor_copy(t_F, p_F)                 # fp32 -> bf16

            # --- BOTH sub-keys minus index in ONE bf16 matmul (exact fp32
            # accum): psum[j,b] = c(b)-j for j<64, p(b)-(j-64) for j>=64 ---
            p_k = ps.tile([2 * PC, B_LOCAL], f32)
            nc.tensor.matmul(p_k, t_W, t_bitsT)
            t_oh = sb.tile([2 * PC, B_LOCAL], bf16)
            nc.vector.tensor_scalar(
                out=t_oh,
                in0=p_k,
                scalar1=0.0,
                scalar2=None,
                op0=mybir.AluOpType.is_equal,
            )

            # --- gather + reduce:  out[b] = F64[p_b, c_b] ---
            p_o1 = ps.tile([PC, B_LOCAL], f32)
            nc.tensor.matmul(
                p_o1, t_F[PC : 2 * PC, :], t_oh[PC : 2 * PC, :])
            t_prod = sb.tile([PC, B_LOCAL], bf16)
            nc.vector.tensor_mul(t_prod, p_o1, t_oh[0:PC, :])
            # column sums, batch transposed onto output partitions: four
            # 1-column matmuls (stationary = a 128-column slice of prod)
            # leave only a [128, 4] PSUM->SBUF copy instead of [1, 512]
            p_out = ps.tile([128, 4], f32)
            for g in range(4):
                nc.tensor.matmul(
                    p_out[:, g : g + 1],
                    t_prod[:, g * 128 : (g + 1) * 128], t_ones)
            t_out = sb.tile([128, 4], f32)
            nc.vector.tensor_copy(t_out, p_out)
            nc.sync.dma_start(out=out, in_=t_out)

    nc.compile()
    return nc


def _get_module():
    nc = _STATE.get("nc")
    if nc is None:
        nc = _build_module()
        _STATE["nc"] = nc
    return nc


def _host_prep(bitstrings, theta, idx_pad):
    """Index bookkeeping + input staging. Returns per-core input maps."""
    import ml_dtypes

    bitstrings = np.asarray(bitstrings)
    theta = np.asarray(theta, dtype=np.float32)
    idx_pad = np.asarray(idx_pad).astype(np.int64)

    # mask[t] = XOR-fold of one-hot bit positions (pad index >= NUM_BITS -> no bit)
    onehots = np.where(idx_pad >= NUM_BITS, 0, np.int64(1) << np.clip(idx_pad, 0, 62))
    masks = np.bitwise_xor.reduce(onehots, axis=1)
    if masks.size and int(masks.max()) >= KEYS:
        raise NotImplementedError(
            "kernel specialized for masks spanning bits 0..11 "
            f"(max mask {int(masks.max())})"
        )
    theta_spread = np.zeros(KEYS, np.float32)
    np.add.at(theta_spread, masks, theta)

    # Stationary key weights.  Column j (j<64) computes c(b)-j, column 64+j
    # computes p(b)-j; row 32 multiplies the constant-ones bit row and
    # carries the -j offsets.
    W = np.zeros((ROWS, 2 * PC), np.float32)
    for k in range(PC_BITS):
        W[k, 0:PC] = float(1 << k)
    for k in range(PC_BITS, ORDER):
        W[k, PC : 2 * PC] = float(1 << (k - PC_BITS))
    W[NUM_BITS, 0:PC] = -np.arange(PC, dtype=np.float32)
    W[NUM_BITS, PC : 2 * PC] = -np.arange(PC, dtype=np.float32)

    h64 = _sylvester(PC)
    pkk = np.zeros((PC, PKK_COLS), np.float32)
    pkk[:, 0:PC] = theta_spread.reshape(PC, PC).T      # thetaT64[md, q]
    pkk[:, PC : 2 * PC] = h64
    pkk[:, 3 * PC : 4 * PC] = h64                      # h64aug = zeros | H64

    base = {"pkk": pkk}

    bits_f = bitstrings.astype(np.float32)
    in_maps = []
    for c in range(N_CORES):
        m = dict(base)
        bbuf = np.ones((ROWS, BB_COLS), np.float32)
        bbuf[:NUM_BITS, 0:B_LOCAL] = bits_f[c * B_LOCAL : (c + 1) * B_LOCAL, :].T
        bbuf[:, B_LOCAL:] = W
        m["bb"] = bbuf.astype(ml_dtypes.bfloat16)
        in_maps.append(m)
    return in_maps


def kernel(bitstrings, theta, idx_pad):
    from concourse.bass_utils import run_bass_kernel_spmd

    in_maps = _host_prep(bitstrings, theta, idx_pad)
    nc = _get_module()
    res = run_bass_kernel_spmd(nc, in_maps, core_ids=list(range(N_CORES)))
    # out[i, g] holds sample b_local = g*128 + i
    out = np.concatenate([np.asarray(r["out"]).T.ravel() for r in res.results])
    return out.astype(np.float32)

